# revision 1
# baseline (speedup 1.0000x reference)
"""Trainium2 Bass kernel for nn_Decoder_4561255269164 (retrieval_knn).

Math: the reference's top-K(8) KNN collapses to min-reductions:
  - backward: weight w=1/sqrt(d) is nonzero only where d equals the row min
    (over kept candidates), so the scatter-add num/den equals
    E_b^T @ [w*rgb, w] with E_b[i,j] = (d2[i,j] == rowmin_i).
  - forward: only the column argmin rows of d2 matter; sumf/cntf =
    E_f^T @ [rgb, 1] with E_f[i,j] = (d2[i,j] <= colmin_j).
  - exact-match (d==0) rows use a separate weight column gated by rowmin==0.

Sharding: targets (N) split across cores (padded to NT*128 rows each).
Pass A computes the MASKED d2 tile-by-tile in bf16 (1 PE cycle/row vs 4 for
fp32).  Row mins use a two-level reduce (elementwise TT-min accumulation at
DVE 2x + one narrow 1x reduce per target tile); a running elementwise min
accumulates column partials (masked colmin == raw colmin for kept j, and only
kept j matter downstream) with odd windows offloaded to the idle GPSIMD
engine.  Column partials collapse across partitions with gpsimd axis=C
reduces, then AllReduce(min) across cores.
Pass B recomputes RAW d2 (bf16 matmuls) with a software-pipelined lag: the
eb/backward side (independent of the colmin AllReduce) runs LAG windows ahead
of the ef/forward side, hiding the collective's latency; the colmin row is
broadcast across partitions once via contract-1 PE matmuls into a persistent
[128, L] tile.  Scatter partials accumulate into [12, L] via indicator
matmuls and AllReduce(add) in bf16, issued per column-chunk so the first
chunk's collective overlaps compute.  The finalize is chunked the same way
(chunk 0 overlaps the tail collective) and the BCE term is computed early.
"""

import numpy as np

import concourse.bass as bass
import concourse.bacc as bacc
import concourse.bass_isa as bass_isa
import concourse.mybir as mybir
import concourse.tile as tile
from concourse import library_config
from concourse.bass_utils import run_bass_kernel_spmd

F32 = mybir.dt.float32
BF16 = mybir.dt.bfloat16
AX = mybir.AxisListType
ALU = mybir.AluOpType
ACTF = mybir.ActivationFunctionType

# geometry
NCORES = 8
L = 16384          # candidates
N = 10000          # targets
NT = 10            # i-tiles of 128 per core (pad 1250 -> 1280)
POINTS_NUM = 8192
BIG = np.float32(1e30)
FWD_EPS = 1.000001  # relative margin for forward colmin match

AT_W = 2048        # pass A window (NGA=4 tile-position groups of 512)
B_W = 1024         # pass B window (NGB=2 groups)
NGA = AT_W // 512
NGB = B_W // 512
NDCH = 4           # nd AllReduce chunks
LAG = 3            # pass-B eb-side lead (windows)


def _build_nc(reps=1, phases=("A", "C", "W", "B", "FIN")):
    nsh = N // NCORES
    npad = NT * 128
    nq = B_W // 512

    nc = bacc.Bacc("TRN2", target_bir_lowering=False, debug=False,
                   num_devices=NCORES)

    c5r = nc.declare_dram_parameter("c5r", [5, L], BF16, isOutput=False)
    c5m = nc.declare_dram_parameter("c5m", [5, L], BF16, isOutput=False)
    t5d = nc.declare_dram_parameter("t5", [5, npad], BF16, isOutput=False)
    trgbd = nc.declare_dram_parameter("trgb", [128, NT * 3], F32, isOutput=False)
    rgbpd = nc.declare_dram_parameter("rgbp", [3, L], F32, isOutput=False)
    keepd = nc.declare_dram_parameter("keepf", [1, L], F32, isOutput=False)
    predd = nc.declare_dram_parameter("predf", [1, L], F32, isOutput=False)
    ktgtd = nc.declare_dram_parameter("ktgt", [1, L], F32, isOutput=False)
    eyed = nc.declare_dram_parameter("eye128", [128, 128], F32, isOutput=False)
    chaind = nc.declare_dram_parameter("chain", [1, 2], F32, isOutput=False)
    outd = nc.declare_dram_parameter("out", [1, 2], F32, isOutput=True)

    rg = [list(range(NCORES))]
    njc = L // B_W           # 16 pass-B windows
    jpc = njc // NDCH        # windows per nd-chunk
    lch = L // NDCH          # columns per nd-chunk

    with tile.TileContext(nc) as tc:
        nc.gpsimd.load_library(library_config.mlp)
        for _rep in range(reps):
            with (
                tc.tile_pool(name="persist", bufs=1) as pp,
                tc.tile_pool(name="dram", bufs=1, space="DRAM") as dp,
            ):
                # ---------------- persistent SBUF loads --------------------
                t5b = pp.tile([(NGA - 1) * 32 + 5, npad], BF16, tag="t5b",
                              name="t5b")
                for g in range(NGA):
                    nc.sync.dma_start(t5b[32 * g:32 * g + 5, :], t5d[:, :])
                c5ab = pp.tile([(NGB - 1) * 32 + 5, L], BF16, tag="c5ab",
                               name="c5ab")
                for g in range(NGB):
                    nc.sync.dma_start(c5ab[32 * g:32 * g + 5, :], c5r[:, :])
                trgb = pp.tile([128, NT * 3], F32, tag="trgb", name="trgb")
                nc.sync.dma_start(trgb[:], trgbd[:, :])
                eye = pp.tile([128, 128], F32, tag="eye", name="eye")
                nc.sync.dma_start(eye[:], eyed[:, :])
                eyeb = pp.tile([128, 128], BF16, tag="eyeb", name="eyeb")
                nc.vector.tensor_copy(eyeb[:], eye[:])
                m2loc = pp.tile([128, L // 128], F32, tag="m2loc")  # [p, jt]

                colpart = pp.tile([128, L], BF16, tag="colpart")
                m_all = pp.tile([128, NT], F32, tag="m_all")   # row mins
                m_relu = pp.tile([128, NT], F32, tag="m_relu")
                wb_all = pp.tile([128, NT * 8], BF16, tag="wb_all")
                wf_all = pp.tile([128, NT * 4], BF16, tag="wf_all")
                lp = L // 128      # finalize plane free width
                rowsbce = pp.tile([128, 1], F32, tag="rowsbce")

                # collectives proved load-safe only with f32 payloads on
                # whole (unsliced) DRAM tensors
                m2_in = dp.tile([L // 128, 128], F32, tag="m2_in")
                m2_loc = dp.tile([L // 128, 128], F32, tag="m2_loc")
                m2_out = dp.tile([1, L], F32, tag="m2_out")
                nd_ins = [dp.tile([12, lch], F32, tag=f"nd_in{ch}",
                                  name=f"nd_in{ch}") for ch in range(NDCH)]
                nd_outs = [dp.tile([12, lch], F32, tag=f"nd_out{ch}",
                                   name=f"nd_out{ch}") for ch in range(NDCH)]

                with tc.tile_pool(name="amask", bufs=1) as pam:
                    c5mb = pam.tile([(NGA - 1) * 32 + 5, L], BF16, tag="c5mb",
                                    name="c5mb")
                    for g in range(NGA):
                        nc.sync.dma_start(c5mb[32 * g:32 * g + 5, :], c5m[:, :])

                    if "A" in phases:
                        # --- Pass A: masked d2; row mins + col partial mins --
                        # window-major (jc outer) so each window's colmin
                        # finalizes (gpsimd cross-partition reduce + DMA) as
                        # soon as its last target tile lands; odd windows'
                        # colmin chains run on the GPSIMD engine to unload
                        # DVE.  The colred DMAs issue from gpsimd to keep the
                        # in-order SP queue free for pass-B stores.
                        with (
                            tc.tile_pool(name="a_d2", bufs=10) as adp,
                            tc.tile_pool(name="a_ps", bufs=2, space="PSUM") as apsp,
                            tc.tile_pool(name="a_r", bufs=1) as arp,
                            tc.tile_pool(name="a_cr", bufs=2) as acr,
                        ):
                            rowps = [arp.tile([128, AT_W], BF16, tag=f"rowp{t}",
                                              name=f"rowp{t}")
                                     for t in range(NT)]
                            nat = L // AT_W
                            for jc in range(nat):
                                wsl = slice(jc * AT_W, (jc + 1) * AT_W)
                                for t in range(NT):
                                    ps = apsp.tile([128, AT_W], F32, tag="aps")
                                    for g in range(NGA):
                                        q0 = g * 512
                                        nc.tensor.matmul(
                                            ps[:, q0:q0 + 512],
                                            lhsT=t5b[32 * g:32 * g + 5,
                                                     t * 128:(t + 1) * 128],
                                            rhs=c5mb[32 * g:32 * g + 5,
                                                     jc * AT_W + q0:
                                                     jc * AT_W + q0 + 512],
                                            start=True, stop=True,
                                            tile_position=(32 * g, 0))
                                    # write the relu straight into its
                                    # first consumer to skip init copies
                                    sl = colpart[:, wsl]
                                    if jc == 0:
                                        dst = rowps[t][:]
                                    elif t == 0:
                                        dst = sl
                                    else:
                                        d2a = adp.tile([128, AT_W], BF16,
                                                       tag="d2a")
                                        dst = d2a[:]
                                    nc.scalar.activation(dst, ps[:], ACTF.Relu)
                                    if jc > 0:
                                        nc.vector.tensor_tensor(
                                            rowps[t][:], rowps[t][:], dst,
                                            op=ALU.min)
                                    if t == 0:
                                        if jc == 0:
                                            nc.vector.tensor_copy(sl, dst)
                                    else:
                                        nc.vector.tensor_tensor(sl, sl, dst,
                                                                op=ALU.min)
                                    if jc == nat - 1:
                                        # final row-min: two folds narrow the
                                        # slow 1x reduce so the chain m_all ->
                                        # weights -> eb is ready at pass B
                                        rp = rowps[t]
                                        nc.vector.tensor_tensor(
                                            rp[:, 0:AT_W // 2],
                                            rp[:, 0:AT_W // 2],
                                            rp[:, AT_W // 2:AT_W], op=ALU.min)
                                        nc.vector.tensor_tensor(
                                            rp[:, 0:AT_W // 4],
                                            rp[:, 0:AT_W // 4],
                                            rp[:, AT_W // 4:AT_W // 2],
                                            op=ALU.min)
                                        nc.vector.tensor_reduce(
                                            m_all[:, t:t + 1],
                                            rp[:, 0:AT_W // 4],
                                            axis=AX.X, op=ALU.min)

                if "W" in phases:
                    # ---------------- weight tiles --------------------------
                    with tc.tile_pool(name="wsmall", bufs=1) as ws:
                        nc.vector.tensor_scalar(m_relu[:], m_all[:], 0.0, None,
                                                op0=ALU.max)
                        msafe = ws.tile([128, NT], F32, tag="msafe")
                        nc.vector.tensor_scalar(msafe[:], m_relu[:], 1e-30, None,
                                                op0=ALU.max)
                        sqm = ws.tile([128, NT], F32, tag="sqm")
                        nc.scalar.activation(sqm[:], msafe[:], ACTF.Sqrt)
                        w0 = ws.tile([128, NT], F32, tag="w0")
                        nc.vector.reciprocal(w0[:], sqm[:])
                        vv = ws.tile([128, NT], F32, tag="vv")
                        nc.vector.tensor_scalar(vv[:], m_relu[:], 0.0, None,
                                                op0=ALU.is_gt)
                        v2 = ws.tile([128, NT], F32, tag="v2")
                        nc.vector.tensor_scalar(v2[:], m_relu[:], 1e29, None,
                                                op0=ALU.is_lt)
                        nc.vector.tensor_tensor(vv[:], vv[:], v2[:], op=ALU.mult)
                        wgt = ws.tile([128, NT], F32, tag="wgt")
                        nc.vector.tensor_tensor(wgt[:], w0[:], vv[:], op=ALU.mult)
                        zz = ws.tile([128, NT], F32, tag="zz")
                        nc.vector.tensor_scalar(zz[:], m_relu[:], 0.0, None,
                                                op0=ALU.is_equal)

                        wbv = wb_all[:].rearrange("p (t k) -> p t k", k=8)
                        wfv = wf_all[:].rearrange("p (t k) -> p t k", k=4)
                        tv = trgb[:].rearrange("p (t k) -> p t k", k=3)
                        wgv = wgt[:].rearrange("p (t o) -> p t o", o=1)
                        zzv = zz[:].rearrange("p (t o) -> p t o", o=1)
                        for c in range(3):
                            nc.vector.tensor_tensor(
                                wbv[:, :, c:c + 1], wgv, tv[:, :, c:c + 1],
                                op=ALU.mult)
                            nc.vector.tensor_tensor(
                                wbv[:, :, 4 + c:5 + c], zzv, tv[:, :, c:c + 1],
                                op=ALU.mult)
                            nc.vector.tensor_copy(wfv[:, :, c:c + 1],
                                                  tv[:, :, c:c + 1])
                        nc.vector.tensor_copy(wbv[:, :, 3:4], wgv)
                        nc.vector.tensor_copy(wbv[:, :, 7:8], zzv)
                        nc.vector.memset(wfv[:, :, 3:4], 1.0)

                if "C" in phases:
                    # --- colmin: bf16 PE transposes + free-axis DVE reduces
                    # (after W so the m_all -> weights -> eb chain clears DVE
                    # first), then AllReduce(min) across cores ---------------
                    with (
                        tc.tile_pool(name="c_ps", bufs=2, space="PSUM") as cps,
                        tc.tile_pool(name="c_ps2", bufs=1, space="PSUM") as cps2,
                        tc.tile_pool(name="c_sb", bufs=1) as csb,
                    ):
                        for jt in range(L // 128):
                            pst = cps.tile([128, 128], BF16, tag="pstb")
                            nc.tensor.transpose(
                                pst[:], colpart[:, jt * 128:(jt + 1) * 128],
                                eyeb[:])
                            nc.vector.tensor_reduce(
                                m2loc[:, jt:jt + 1], pst[:], axis=AX.X,
                                op=ALU.min)
                        pst2 = cps2.tile([128, 128], F32, tag="pst2")
                        nc.tensor.transpose(pst2[:], m2loc[:], eye[:])
                        m2t = csb.tile([128, 128], F32, tag="m2t")
                        nc.vector.tensor_copy(m2t[:], pst2[:])
                        nc.sync.dma_start(m2_in[:, :], m2t[:])
                    if NCORES > 1:
                        nc.gpsimd.collective_compute(
                            "AllReduce", ALU.min, replica_groups=rg,
                            ins=[m2_in.opt()], outs=[m2_out.opt()])
                    else:
                        nc.sync.dma_start(m2_out[:, :], m2_in[:, :])

                if "FIN" in phases:
                    # ---- BCE term early: relu(p) - p*t + softplus(-|p|) ----
                    with tc.tile_pool(name="finE", bufs=1) as fe:
                        predf = fe.tile([128, lp], F32, tag="predf", name="predf")
                        nc.sync.dma_start(
                            predf[:], predd[0, :].rearrange("(p q) -> p q", p=128))
                        ktgt = fe.tile([128, lp], F32, tag="ktgt", name="ktgt")
                        nc.sync.dma_start(
                            ktgt[:], ktgtd[0, :].rearrange("(p q) -> p q", p=128))
                        bce = fe.tile([128, lp], F32, tag="bce")
                        nc.scalar.activation(bce[:], predf[:], ACTF.Relu)
                        pt = fe.tile([128, lp], F32, tag="pt")
                        nc.vector.tensor_tensor(pt[:], predf[:], ktgt[:],
                                                op=ALU.mult)
                        nc.vector.tensor_tensor(bce[:], bce[:], pt[:],
                                                op=ALU.subtract)
                        ap_ = fe.tile([128, lp], F32, tag="ap_")
                        nc.scalar.activation(ap_[:], predf[:], ACTF.Abs)
                        en = fe.tile([128, lp], F32, tag="en")
                        nc.scalar.activation(en[:], ap_[:], ACTF.Exp, scale=-1.0)
                        sp = fe.tile([128, lp], F32, tag="sp")
                        nc.scalar.activation(sp[:], en[:], ACTF.Ln, bias=1.0)
                        nc.vector.tensor_tensor(bce[:], bce[:], sp[:], op=ALU.add)
                        nc.vector.tensor_reduce(rowsbce[:], bce[:], axis=AX.X,
                                                op=ALU.add)

                if "B" in phases:
                    # --- Pass B: raw d2; eb vs row min, ef vs LOCAL colmin --
                    # ef compares against this core's own colmin (no wait on
                    # the colmin AllReduce); non-winning cores' forward sums
                    # are zeroed by an equality mask (local == global colmin)
                    # right before each chunk's AllReduce(add).  The colmin
                    # row broadcasts across partitions via contract-1 matmuls
                    # pipelined one window ahead.
                    prow = lch // lp
                    with (
                        tc.tile_pool(name="b_pers", bufs=1) as pbp,
                        tc.tile_pool(name="b_m2b", bufs=2) as bm2b,
                        tc.tile_pool(name="b_d2", bufs=(LAG + 1) * NT + 4) as bd2,
                        tc.tile_pool(name="b_e", bufs=4) as bep,
                        tc.tile_pool(name="b_nd", bufs=4) as bnd,
                        tc.tile_pool(name="b_msk", bufs=2) as bmsk,
                        tc.tile_pool(name="b_psd", bufs=2, space="PSUM") as bpsd,
                        tc.tile_pool(name="b_acc", bufs=1, space="PSUM") as baccp,
                    ):


                        def mask_fwd(ch):
                            # zero sf/cntf where this core's colmin isn't the
                            # global winner (ties keep all winners, matching
                            # the old <=-with-ties semantics)
                            csl = slice(ch * lch, (ch + 1) * lch)
                            m2l = bmsk.tile([prow, lp], F32, tag="m2l")
                            nc.sync.dma_start(
                                m2l[:], m2_loc[ch * prow:(ch + 1) * prow, :])
                            pks = []
                            for k in range(8, 12):
                                pk = bmsk.tile([prow, lp], F32, tag=f"pk{k}")
                                nc.sync.dma_start(
                                    pk[:], nd_ins[ch][k, :].rearrange(
                                        "(p q) -> p q", p=prow))
                                pks.append(pk)
                            m2g = bmsk.tile([prow, lp], F32, tag="m2g")
                            nc.sync.dma_start(
                                m2g[:],
                                m2_out[0, csl].rearrange("(p q) -> p q", p=prow))
                            msk = bmsk.tile([prow, lp], F32, tag="msk")
                            nc.vector.tensor_tensor(msk[:], m2l[:], m2g[:],
                                                    op=ALU.is_equal)
                            for k in range(8, 12):
                                pk = pks[k - 8]
                                nc.vector.tensor_tensor(pk[:], pk[:], msk[:],
                                                        op=ALU.mult)
                                nc.sync.dma_start(
                                    nd_ins[ch][k, :].rearrange("(p q) -> p q",
                                                               p=prow), pk[:])

                        def reduce_chunk(ch):
                            if "NOAR" in phases:
                                pass
                            elif NCORES > 1:
                                nc.gpsimd.collective_compute(
                                    "AllReduce", ALU.add, replica_groups=rg,
                                    ins=[nd_ins[ch].opt()],
                                    outs=[nd_outs[ch].opt()])
                            else:
                                nc.sync.dma_start(nd_outs[ch][:, :],
                                                  nd_ins[ch][:, :])

                        def sub_i(jc):
                            # eb/backward side: independent of the colmin
                            # AllReduce, runs LAG windows ahead
                            accb = [baccp.tile([8, 512], F32, tag=f"accb{q}",
                                               name=f"accb{q}")
                                    for q in range(nq)]
                            d2bs = []
                            for t in range(NT):
                                psd = bpsd.tile([128, B_W], F32, tag="psd")
                                for g in range(NGB):
                                    q0 = g * 512
                                    nc.tensor.matmul(
                                        psd[:, q0:q0 + 512],
                                        lhsT=t5b[32 * g:32 * g + 5,
                                                 t * 128:(t + 1) * 128],
                                        rhs=c5ab[32 * g:32 * g + 5,
                                                 jc * B_W + q0:
                                                 jc * B_W + q0 + 512],
                                        start=True, stop=True,
                                        tile_position=(32 * g, 0))
                                d2b = bd2.tile([128, B_W], BF16, tag="d2b")
                                nc.scalar.activation(d2b[:], psd[:], ACTF.Relu)
                                d2bs.append(d2b)
                            for t in range(NT):
                                eb = bep.tile([128, B_W], BF16, tag="eb")
                                nc.vector.tensor_scalar(eb[:], d2bs[t][:],
                                                        m_relu[:, t:t + 1],
                                                        None, op0=ALU.is_equal)
                                for q in range(nq):
                                    nc.tensor.matmul(
                                        accb[q][:, :],
                                        lhsT=wb_all[:, t * 8:(t + 1) * 8],
                                        rhs=eb[:, q * 512:(q + 1) * 512],
                                        start=(t == 0), stop=(t == NT - 1))
                            ch, col = divmod(jc, jpc)
                            for q in range(nq):
                                j0 = col * B_W + q * 512
                                ndb = bnd.tile([8, 512], F32, tag="ndb")
                                nc.scalar.copy(ndb[:], accb[q][:, :])
                                nc.sync.dma_start(nd_ins[ch][0:8, j0:j0 + 512],
                                                  ndb[:])
                            return d2bs

                        def sub_ii(jc, d2bs):
                            # ef/forward side vs global colmin (needs the
                            # colmin AllReduce, hidden behind sub_i's lead)
                            m2w = bm2b.tile([1, B_W], F32, tag="m2w")
                            nc.sync.dma_start(
                                m2w[:], m2_out[:, jc * B_W:(jc + 1) * B_W])
                            m2wb = bm2b.tile([1, B_W], BF16, tag="m2wb")
                            nc.vector.tensor_copy(m2wb[:], m2w[:])
                            m2sl = bm2b.tile([128, B_W], BF16, tag="m2b")
                            nc.gpsimd.partition_broadcast(m2sl[:], m2wb[:])
                            accf = [baccp.tile([4, 512], F32, tag=f"accf{q}",
                                               name=f"accf{q}")
                                    for q in range(nq)]
                            for t in range(NT):
                                ef = bep.tile([128, B_W], BF16, tag="ef")
                                nc.vector.tensor_tensor(ef[:], d2bs[t][:],
                                                        m2sl[:], op=ALU.is_le)
                                for q in range(nq):
                                    nc.tensor.matmul(
                                        accf[q][:, :],
                                        lhsT=wf_all[:, t * 4:(t + 1) * 4],
                                        rhs=ef[:, q * 512:(q + 1) * 512],
                                        start=(t == 0), stop=(t == NT - 1))
                            ch, col = divmod(jc, jpc)
                            for q in range(nq):
                                j0 = col * B_W + q * 512
                                ndf = bnd.tile([4, 512], F32, tag="ndf")
                                nc.vector.tensor_copy(ndf[:], accf[q][:, :])
                                nc.sync.dma_start(nd_ins[ch][8:12, j0:j0 + 512],
                                                  ndf[:])
                            if (jc + 1) % jpc == 0:
                                reduce_chunk(jc // jpc)

                        pend = {}
                        for jc in range(njc):
                            pend[jc] = sub_i(jc)
                            if jc >= LAG:
                                sub_ii(jc - LAG, pend.pop(jc - LAG))
                        for jc in range(njc - LAG, njc):
                            sub_ii(jc, pend.pop(jc))

                if "FIN" in phases:
                    # ---- finalize, chunked by nd AllReduce chunk (chunk 0
                    # runs while chunk 1's collective is in flight) ----------
                    prow = lch // lp   # plane partitions per nd chunk
                    with (
                        tc.tile_pool(name="fin", bufs=1) as fp,
                        tc.tile_pool(name="fin_ps", bufs=1, space="PSUM") as fps,
                    ):
                        accp = fp.tile([prow, NDCH], F32, tag="accp")
                        for ch in range(NDCH):
                            j0 = ch * lch

                            def plane(dram_row, tg):
                                tl = fp.tile([prow, lp], F32, tag=tg,
                                             name=f"{tg}_{ch}")
                                nc.sync.dma_start(
                                    tl[:], dram_row.rearrange("(p q) -> p q",
                                                              p=prow))
                                return tl

                            def plane_nd(k, tg):
                                tl = fp.tile([prow, lp], F32, tag=tg,
                                             name=f"{tg}_{ch}")
                                nc.sync.dma_start(
                                    tl[:], nd_outs[ch][k, :].rearrange(
                                        "(p q) -> p q", p=prow))
                                return tl

                            rgbp = [plane(rgbpd[k, j0:j0 + lch], f"rgb{k}")
                                    for k in range(3)]
                            keepf = plane(keepd[0, j0:j0 + lch], "keepf")
                            nd = [plane_nd(k, f"nd{k}") for k in range(12)]

                            num, den = nd[0:3], nd[3]
                            s0, cnt0 = nd[4:7], nd[7]
                            sf, cntf = nd[8:11], nd[11]

                            _cnt = [0]

                            def newt():
                                _cnt[0] += 1
                                return fp.tile([prow, lp], F32,
                                               tag=f"fin{_cnt[0]}",
                                               name=f"fin{_cnt[0]}_{ch}")

                            dsafe = newt()
                            nc.vector.tensor_scalar(dsafe[:], den[:], 0.0, None,
                                                    op0=ALU.is_equal)
                            nc.vector.tensor_tensor(dsafe[:], dsafe[:], den[:],
                                                    op=ALU.add)
                            rden = newt()
                            nc.vector.reciprocal(rden[:], dsafe[:])
                            c0safe = newt()
                            nc.vector.tensor_scalar(c0safe[:], cnt0[:], 0.0, None,
                                                    op0=ALU.is_equal)
                            nc.vector.tensor_tensor(c0safe[:], c0safe[:], cnt0[:],
                                                    op=ALU.add)
                            rcnt0 = newt()
                            nc.vector.reciprocal(rcnt0[:], c0safe[:])
                            rcntf = newt()
                            nc.vector.reciprocal(rcntf[:], cntf[:])

                            mden = fp.tile([prow, lp], mybir.dt.int32,
                                           tag="mden", name=f"mden_{ch}")
                            nc.vector.tensor_scalar(mden[:], den[:], 0.0, None,
                                                    op0=ALU.not_equal)
                            mz = fp.tile([prow, lp], mybir.dt.int32, tag="mz",
                                         name=f"mz_{ch}")
                            nc.vector.tensor_scalar(mz[:], cnt0[:], 0.0, None,
                                                    op0=ALU.is_gt)

                            acc = newt()
                            nc.vector.memset(acc[:], 0.0)
                            for c in range(3):
                                rec = newt()
                                nc.vector.tensor_tensor(rec[:], sf[c][:],
                                                        rcntf[:], op=ALU.mult)
                                tmp = newt()
                                nc.vector.tensor_tensor(tmp[:], num[c][:],
                                                        rden[:], op=ALU.mult)
                                nc.vector.copy_predicated(rec[:], mden[:], tmp[:])
                                nc.vector.tensor_tensor(tmp[:], s0[c][:],
                                                        rcnt0[:], op=ALU.mult)
                                nc.vector.copy_predicated(rec[:], mz[:], tmp[:])
                                diff = newt()
                                nc.vector.tensor_tensor(diff[:], rgbp[c][:],
                                                        rec[:], op=ALU.subtract)
                                ad = newt()
                                nc.scalar.activation(ad[:], diff[:], ACTF.Abs)
                                nc.vector.tensor_tensor(acc[:], acc[:], ad[:],
                                                        op=ALU.add)
                            nc.vector.tensor_tensor(acc[:], acc[:], keepf[:],
                                                    op=ALU.mult)
                            nc.vector.tensor_reduce(accp[:, ch:ch + 1], acc[:],
                                                    axis=AX.X, op=ALU.add)

                        onescol = fp.tile([128, 1], F32, tag="onescol")
                        nc.vector.memset(onescol[:], 1.0)
                        ps_a = fps.tile([1, 1], F32, tag="ps_a")
                        nc.tensor.matmul(ps_a[:], lhsT=onescol[:],
                                         rhs=rowsbce[:], start=True, stop=True)
                        ps_b = fps.tile([1, NDCH], F32, tag="ps_b")
                        nc.tensor.matmul(ps_b[:], lhsT=onescol[0:prow, :],
                                         rhs=accp[:], start=True, stop=True)
                        chsb = fp.tile([1, 2], F32, tag="chsb")
                        nc.sync.dma_start(chsb[:], chaind[:, :])
                        nc.vector.tensor_scalar(chsb[:], chsb[:], 0.0, None,
                                                op0=ALU.mult)
                        outsb = fp.tile([1, 2], F32, tag="outsb")
                        nc.scalar.copy(outsb[:, 0:1], ps_a[:])
                        nc.vector.tensor_reduce(outsb[:, 1:2], ps_b[:],
                                                axis=AX.X, op=ALU.add)
                        nc.vector.tensor_tensor(outsb[:], outsb[:], chsb[:],
                                                op=ALU.add)
                        nc.sync.dma_start(outd[:, :], outsb[:])

    nc.compile()
    return nc


def _host_prep(pred_F, cand_xyz, cand_rgb, tgt_xyz, tgt_rgb, keep_target,
               points_num):
    bf16 = mybir.dt.np(BF16)
    nsh = N // NCORES
    npad = NT * 128
    pred = np.ascontiguousarray(np.asarray(pred_F, np.float32))
    cxyz = np.ascontiguousarray(np.asarray(cand_xyz, np.float32))
    crgb = np.ascontiguousarray(np.asarray(cand_rgb, np.float32))
    txyz = np.ascontiguousarray(np.asarray(tgt_xyz, np.float32))
    trgb_np = np.ascontiguousarray(np.asarray(tgt_rgb, np.float32))
    ktgt = np.asarray(keep_target).astype(np.float32)

    # keep mask (exact reference semantics, f32)
    p8 = pred.reshape(-1, 8)
    rows = np.arange(p8.shape[0])
    ilm = np.zeros(p8.shape, dtype=bool)
    ilm[rows, np.argmax(p8, axis=1)] = True
    ilm = ilm.reshape(-1)
    k = L - int(points_num)
    vals = np.where(ilm, np.inf, pred)
    thr = np.sort(vals)[k - 1]
    keep = (pred > thr) | ilm

    b2 = np.sum(cxyz * cxyz, axis=1, dtype=np.float32).astype(np.float32)
    b2m = np.where(keep, b2, BIG).astype(np.float32)
    ones = np.ones(L, np.float32)
    c5r_np = np.stack([cxyz[:, 0], cxyz[:, 1], cxyz[:, 2], ones, b2])
    c5m_np = np.stack([cxyz[:, 0], cxyz[:, 1], cxyz[:, 2], ones, b2m])
    c5r_np = np.ascontiguousarray(c5r_np.astype(bf16))
    c5m_np = np.ascontiguousarray(c5m_np.astype(bf16))

    a2 = np.sum(txyz * txyz, axis=1, dtype=np.float32).astype(np.float32)

    t5_cores, trgb_cores = [], []
    for c in range(NCORES):
        sl = slice(c * nsh, (c + 1) * nsh)
        t5 = np.zeros((5, npad), np.float32)
        t5[3, :] = BIG     # pad rows: s = 1e30 everywhere
        t5[4, :] = 1.0
        t5[0, :nsh] = -2.0 * txyz[sl, 0]
        t5[1, :nsh] = -2.0 * txyz[sl, 1]
        t5[2, :nsh] = -2.0 * txyz[sl, 2]
        t5[3, :nsh] = a2[sl]
        tr = np.zeros((npad, 3), np.float32)
        tr[:nsh] = trgb_np[sl]
        # [p, t*3+c] layout: target i_local = t*128 + p
        trc = tr.reshape(NT, 128, 3).transpose(1, 0, 2).reshape(128, NT * 3)
        t5_cores.append(np.ascontiguousarray(t5.astype(bf16)))
        trgb_cores.append(np.ascontiguousarray(trc))

    rgbp = np.ascontiguousarray((crgb * np.float32(255.0)).T.astype(np.float32))
    keepf = keep.astype(np.float32).reshape(1, L)
    eye = np.eye(128, dtype=np.float32)

    common = dict(c5r=c5r_np, c5m=c5m_np, rgbp=rgbp,
                  keepf=keepf, predf=pred.reshape(1, L),
                  ktgt=ktgt.reshape(1, L), eye128=eye,
                  chain=np.zeros((1, 2), np.float32))
    in_maps = [dict(common, t5=t5_cores[c], trgb=trgb_cores[c])
               for c in range(NCORES)]
    return in_maps


_CACHE = {}


def kernel(pred_F, cand_xyz, cand_rgb, tgt_xyz, tgt_rgb, keep_target,
           points_num=8192, **_ignored):
    in_maps = _host_prep(pred_F, cand_xyz, cand_rgb, tgt_xyz, tgt_rgb,
                         keep_target, points_num)
    if "nc" not in _CACHE:
        _CACHE["nc"] = _build_nc()
    res = run_bass_kernel_spmd(_CACHE["nc"], in_maps,
                               core_ids=list(range(NCORES)))
    return np.asarray(res.results[0]["out"], np.float32).reshape(2)


if __name__ == "__main__":
    import reference as R
    inputs = R.setup_inputs()
    inputs = {kk: np.asarray(vv) if not np.isscalar(vv) else vv
              for kk, vv in inputs.items()}
    out = kernel(**inputs)
    print("kernel out:", out)



# revision 2
# speedup vs baseline: 1.8098x; 1.8098x over previous
"""Trainium2 Bass kernel for nn_Decoder_4561255269164 (retrieval_knn).

Math: the reference's top-K(8) KNN collapses to min-reductions:
  - backward: weight w=1/sqrt(d) is nonzero only where d equals the row min
    (over kept candidates), so the scatter-add num/den equals
    E_b^T @ [w*rgb, w] with E_b[i,j] = (d2[i,j] == rowmin_i).
  - forward: only the column argmin rows of d2 matter; sumf/cntf =
    E_f^T @ [rgb, 1] with E_f[i,j] = (d2[i,j] <= colmin_j).
  - exact-match (d==0) rows use a separate weight column gated by rowmin==0.

Key reduction: every path above only involves KEPT candidates (non-kept are
masked to BIG in the KNN and excluded from the loss), and #kept <= L -
(L - points_num) = 8192 by construction, so the host compacts candidates to a
fixed LK=8192 columns (pad columns get b2=BIG so they never win a min and
keepf=0 so they never reach the loss).  This halves all device work.

Sharding: targets (N) split across cores (padded to NT*128 rows each).
Pass A computes d2 tile-by-tile in bf16 (tile_position-packed contract-5
matmuls, 4x concurrent).  Row mins use a two-level reduce (elementwise TT-min
accumulation at DVE 2x + folds + one narrow 1x reduce per target tile); a
running elementwise min accumulates column partials into colpart.  Column
partials collapse across partitions with PE transposes + DVE reduces, then
AllReduce(min) across cores.
Pass B recomputes d2 (bf16 matmuls) with a software-pipelined lag: the
eb/backward side (independent of the colmin AllReduce) runs LAG windows ahead
of the ef/forward side, hiding the collective's latency; the colmin row is
broadcast across partitions via gpsimd per window.  Scatter partials
accumulate into [12, LK] via indicator matmuls and AllReduce(add), issued per
column-chunk so the first chunk's collective overlaps compute.  The finalize
is chunked the same way and the BCE term (over the FULL 16384 pred_F) is
computed early.
"""

import numpy as np

import concourse.bass as bass
import concourse.bacc as bacc
import concourse.bass_isa as bass_isa
import concourse.mybir as mybir
import concourse.tile as tile
from concourse import library_config
from concourse.bass_utils import run_bass_kernel_spmd

F32 = mybir.dt.float32
BF16 = mybir.dt.bfloat16
AX = mybir.AxisListType
ALU = mybir.AluOpType
ACTF = mybir.ActivationFunctionType

# geometry
NCORES = 8
LBCE = 16384       # full candidate count (BCE over pred_F / keep_target)
LK = 8192          # compacted kept candidates (#kept <= 8192 always)
N = 10000          # targets
NT = 10            # i-tiles of 128 per core (pad 1250 -> 1280)
BIG = np.float32(1e30)

AT_W = 2048        # pass A window (NGA=4 tile-position groups of 512)
B_W = 1024         # pass B window (NGB=2 groups)
NGA = AT_W // 512
NGB = B_W // 512
NDCH = 4           # nd AllReduce chunks
LAG = 3            # pass-B eb-side lead (windows)


def _build_nc(reps=1, phases=("A", "C", "W", "B", "FIN")):
    npad = NT * 128
    nq = B_W // 512

    nc = bacc.Bacc("TRN2", target_bir_lowering=False, debug=False,
                   num_devices=NCORES)

    c5r = nc.declare_dram_parameter("c5r", [5, LK], BF16, isOutput=False)
    t5d = nc.declare_dram_parameter("t5", [5, npad], BF16, isOutput=False)
    trgbd = nc.declare_dram_parameter("trgb", [128, NT * 3], F32, isOutput=False)
    rgbpd = nc.declare_dram_parameter("rgbp", [3, LK], F32, isOutput=False)
    keepd = nc.declare_dram_parameter("keepf", [1, LK], F32, isOutput=False)
    predd = nc.declare_dram_parameter("predf", [1, LBCE], F32, isOutput=False)
    ktgtd = nc.declare_dram_parameter("ktgt", [1, LBCE], F32, isOutput=False)
    eyed = nc.declare_dram_parameter("eye128", [128, 128], F32, isOutput=False)
    chaind = nc.declare_dram_parameter("chain", [1, 2], F32, isOutput=False)
    outd = nc.declare_dram_parameter("out", [1, 2], F32, isOutput=True)

    rg = [list(range(NCORES))]
    njc = LK // B_W          # 8 pass-B windows
    jpc = njc // NDCH        # windows per nd-chunk
    lch = LK // NDCH         # columns per nd-chunk
    lpb = LBCE // 128        # BCE plane free width
    lpf = LK // 128          # finalize plane free width
    nct = LK // 128          # colmin transpose tiles

    with tile.TileContext(nc) as tc:
        nc.gpsimd.load_library(library_config.mlp)
        for _rep in range(reps):
            with (
                tc.tile_pool(name="persist", bufs=1) as pp,
                tc.tile_pool(name="dram", bufs=1, space="DRAM") as dp,
            ):
                # ---------------- persistent SBUF loads --------------------
                t5b = pp.tile([(NGA - 1) * 32 + 5, npad], BF16, tag="t5b",
                              name="t5b")
                for g in range(NGA):
                    nc.sync.dma_start(t5b[32 * g:32 * g + 5, :], t5d[:, :])
                # one candidate array serves pass A (groups 0..3) and pass B
                # (groups 0..1): compacted kept candidates are unmasked
                c5b = pp.tile([(NGA - 1) * 32 + 5, LK], BF16, tag="c5b",
                              name="c5b")
                for g in range(NGA):
                    nc.sync.dma_start(c5b[32 * g:32 * g + 5, :], c5r[:, :])
                trgb = pp.tile([128, NT * 3], F32, tag="trgb", name="trgb")
                nc.sync.dma_start(trgb[:], trgbd[:, :])
                eye = pp.tile([128, 128], F32, tag="eye", name="eye")
                nc.sync.dma_start(eye[:], eyed[:, :])
                eyeb = pp.tile([128, 128], BF16, tag="eyeb", name="eyeb")
                nc.vector.tensor_copy(eyeb[:], eye[:])
                m2loc = pp.tile([128, nct], F32, tag="m2loc")  # [p, jt]

                colpart = pp.tile([128, LK], BF16, tag="colpart")
                m_all = pp.tile([128, NT], F32, tag="m_all")   # row mins
                m_relu = pp.tile([128, NT], F32, tag="m_relu")
                wb_all = pp.tile([128, NT * 8], BF16, tag="wb_all")
                wf_all = pp.tile([128, NT * 4], BF16, tag="wf_all")
                rowsbce = pp.tile([128, 1], F32, tag="rowsbce")

                # collectives proved load-safe only with f32 payloads on
                # whole (unsliced) DRAM tensors
                m2_in = dp.tile([nct, 128], F32, tag="m2_in")
                m2_out = dp.tile([1, LK], F32, tag="m2_out")
                nd_ins = [dp.tile([12, lch], F32, tag=f"nd_in{ch}",
                                  name=f"nd_in{ch}") for ch in range(NDCH)]
                nd_outs = [dp.tile([12, lch], F32, tag=f"nd_out{ch}",
                                   name=f"nd_out{ch}") for ch in range(NDCH)]

                if "A" in phases:
                    # --- Pass A: d2; row mins + col partial mins -----------
                    # window-major (jc outer) so each window's colmin
                    # finalizes as soon as its last target tile lands.
                    with (
                        tc.tile_pool(name="a_d2", bufs=10) as adp,
                        tc.tile_pool(name="a_ps", bufs=2, space="PSUM") as apsp,
                        tc.tile_pool(name="a_r", bufs=1) as arp,
                    ):
                        rowps = [arp.tile([128, AT_W], BF16, tag=f"rowp{t}",
                                          name=f"rowp{t}")
                                 for t in range(NT)]
                        nat = LK // AT_W
                        for jc in range(nat):
                            wsl = slice(jc * AT_W, (jc + 1) * AT_W)
                            for t in range(NT):
                                ps = apsp.tile([128, AT_W], F32, tag="aps")
                                for g in range(NGA):
                                    q0 = g * 512
                                    nc.tensor.matmul(
                                        ps[:, q0:q0 + 512],
                                        lhsT=t5b[32 * g:32 * g + 5,
                                                 t * 128:(t + 1) * 128],
                                        rhs=c5b[32 * g:32 * g + 5,
                                                jc * AT_W + q0:
                                                jc * AT_W + q0 + 512],
                                        start=True, stop=True,
                                        tile_position=(32 * g, 0))
                                # write the relu straight into its
                                # first consumer to skip init copies
                                sl = colpart[:, wsl]
                                if jc == 0:
                                    dst = rowps[t][:]
                                elif t == 0:
                                    dst = sl
                                else:
                                    d2a = adp.tile([128, AT_W], BF16,
                                                   tag="d2a")
                                    dst = d2a[:]
                                nc.scalar.activation(dst, ps[:], ACTF.Relu)
                                if jc > 0:
                                    nc.vector.tensor_tensor(
                                        rowps[t][:], rowps[t][:], dst,
                                        op=ALU.min)
                                if t == 0:
                                    if jc == 0:
                                        nc.vector.tensor_copy(sl, dst)
                                else:
                                    nc.vector.tensor_tensor(sl, sl, dst,
                                                            op=ALU.min)
                                if jc == nat - 1:
                                    # final row-min: two folds narrow the
                                    # slow 1x reduce so the chain m_all ->
                                    # weights -> eb is ready at pass B
                                    rp = rowps[t]
                                    nc.vector.tensor_tensor(
                                        rp[:, 0:AT_W // 2],
                                        rp[:, 0:AT_W // 2],
                                        rp[:, AT_W // 2:AT_W], op=ALU.min)
                                    nc.vector.tensor_tensor(
                                        rp[:, 0:AT_W // 4],
                                        rp[:, 0:AT_W // 4],
                                        rp[:, AT_W // 4:AT_W // 2],
                                        op=ALU.min)
                                    nc.vector.tensor_reduce(
                                        m_all[:, t:t + 1],
                                        rp[:, 0:AT_W // 4],
                                        axis=AX.X, op=ALU.min)

                if "W" in phases:
                    # ---------------- weight tiles --------------------------
                    with tc.tile_pool(name="wsmall", bufs=1) as ws:
                        nc.vector.tensor_scalar(m_relu[:], m_all[:], 0.0, None,
                                                op0=ALU.max)
                        msafe = ws.tile([128, NT], F32, tag="msafe")
                        nc.vector.tensor_scalar(msafe[:], m_relu[:], 1e-30, None,
                                                op0=ALU.max)
                        sqm = ws.tile([128, NT], F32, tag="sqm")
                        nc.scalar.activation(sqm[:], msafe[:], ACTF.Sqrt)
                        w0 = ws.tile([128, NT], F32, tag="w0")
                        nc.vector.reciprocal(w0[:], sqm[:])
                        vv = ws.tile([128, NT], F32, tag="vv")
                        nc.vector.tensor_scalar(vv[:], m_relu[:], 0.0, None,
                                                op0=ALU.is_gt)
                        v2 = ws.tile([128, NT], F32, tag="v2")
                        nc.vector.tensor_scalar(v2[:], m_relu[:], 1e29, None,
                                                op0=ALU.is_lt)
                        nc.vector.tensor_tensor(vv[:], vv[:], v2[:], op=ALU.mult)
                        wgt = ws.tile([128, NT], F32, tag="wgt")
                        nc.vector.tensor_tensor(wgt[:], w0[:], vv[:], op=ALU.mult)
                        zz = ws.tile([128, NT], F32, tag="zz")
                        nc.vector.tensor_scalar(zz[:], m_relu[:], 0.0, None,
                                                op0=ALU.is_equal)

                        wbv = wb_all[:].rearrange("p (t k) -> p t k", k=8)
                        wfv = wf_all[:].rearrange("p (t k) -> p t k", k=4)
                        tv = trgb[:].rearrange("p (t k) -> p t k", k=3)
                        wgv = wgt[:].rearrange("p (t o) -> p t o", o=1)
                        zzv = zz[:].rearrange("p (t o) -> p t o", o=1)
                        for c in range(3):
                            nc.vector.tensor_tensor(
                                wbv[:, :, c:c + 1], wgv, tv[:, :, c:c + 1],
                                op=ALU.mult)
                            nc.vector.tensor_tensor(
                                wbv[:, :, 4 + c:5 + c], zzv, tv[:, :, c:c + 1],
                                op=ALU.mult)
                            nc.vector.tensor_copy(wfv[:, :, c:c + 1],
                                                  tv[:, :, c:c + 1])
                        nc.vector.tensor_copy(wbv[:, :, 3:4], wgv)
                        nc.vector.tensor_copy(wbv[:, :, 7:8], zzv)
                        nc.vector.memset(wfv[:, :, 3:4], 1.0)

                if "C" in phases:
                    # --- colmin: bf16 PE transposes + free-axis DVE reduces
                    # (after W so the m_all -> weights -> eb chain clears DVE
                    # first), then AllReduce(min) across cores ---------------
                    with (
                        tc.tile_pool(name="c_ps", bufs=2, space="PSUM") as cps,
                        tc.tile_pool(name="c_ps2", bufs=1, space="PSUM") as cps2,
                        tc.tile_pool(name="c_sb", bufs=1) as csb,
                    ):
                        for jt in range(nct):
                            pst = cps.tile([128, 128], BF16, tag="pstb")
                            nc.tensor.transpose(
                                pst[:], colpart[:, jt * 128:(jt + 1) * 128],
                                eyeb[:])
                            nc.vector.tensor_reduce(
                                m2loc[:, jt:jt + 1], pst[:], axis=AX.X,
                                op=ALU.min)
                        pst2 = cps2.tile([nct, 128], F32, tag="pst2")
                        nc.tensor.transpose(pst2[:], m2loc[:], eye[:])
                        m2t = csb.tile([nct, 128], F32, tag="m2t")
                        nc.vector.tensor_copy(m2t[:], pst2[:])
                        nc.sync.dma_start(m2_in[:, :], m2t[:])
                    if NCORES > 1:
                        nc.gpsimd.collective_compute(
                            "AllReduce", ALU.min, replica_groups=rg,
                            ins=[m2_in.opt()], outs=[m2_out.opt()])
                    else:
                        nc.sync.dma_start(m2_out[:, :], m2_in[:, :])

                if "FIN" in phases:
                    # ---- BCE term early: relu(p) - p*t + softplus(-|p|) ----
                    with tc.tile_pool(name="finE", bufs=1) as fe:
                        predf = fe.tile([128, lpb], F32, tag="predf", name="predf")
                        nc.sync.dma_start(
                            predf[:], predd[0, :].rearrange("(p q) -> p q", p=128))
                        ktgt = fe.tile([128, lpb], F32, tag="ktgt", name="ktgt")
                        nc.sync.dma_start(
                            ktgt[:], ktgtd[0, :].rearrange("(p q) -> p q", p=128))
                        bce = fe.tile([128, lpb], F32, tag="bce")
                        nc.scalar.activation(bce[:], predf[:], ACTF.Relu)
                        pt = fe.tile([128, lpb], F32, tag="pt")
                        nc.vector.tensor_tensor(pt[:], predf[:], ktgt[:],
                                                op=ALU.mult)
                        nc.vector.tensor_tensor(bce[:], bce[:], pt[:],
                                                op=ALU.subtract)
                        ap_ = fe.tile([128, lpb], F32, tag="ap_")
                        nc.scalar.activation(ap_[:], predf[:], ACTF.Abs)
                        en = fe.tile([128, lpb], F32, tag="en")
                        nc.scalar.activation(en[:], ap_[:], ACTF.Exp, scale=-1.0)
                        sp = fe.tile([128, lpb], F32, tag="sp")
                        nc.scalar.activation(sp[:], en[:], ACTF.Ln, bias=1.0)
                        nc.vector.tensor_tensor(bce[:], bce[:], sp[:], op=ALU.add)
                        nc.vector.tensor_reduce(rowsbce[:], bce[:], axis=AX.X,
                                                op=ALU.add)

                if "B" in phases:
                    # --- Pass B: d2 again; eb vs row min, ef vs colmin -----
                    # The eb/backward side (independent of the colmin
                    # AllReduce) runs LAG windows ahead of the ef/forward
                    # side, hiding the collective's latency.  The colmin row
                    # broadcasts across partitions via gpsimd per window.
                    with (
                        tc.tile_pool(name="b_m2b", bufs=2) as bm2b,
                        tc.tile_pool(name="b_d2", bufs=(LAG + 1) * NT + 4) as bd2,
                        tc.tile_pool(name="b_e", bufs=4) as bep,
                        tc.tile_pool(name="b_nd", bufs=4) as bnd,
                        tc.tile_pool(name="b_psd", bufs=2, space="PSUM") as bpsd,
                        tc.tile_pool(name="b_acc", bufs=1, space="PSUM") as baccp,
                    ):

                        def reduce_chunk(ch):
                            if "NOAR" in phases:
                                pass
                            elif NCORES > 1:
                                nc.gpsimd.collective_compute(
                                    "AllReduce", ALU.add, replica_groups=rg,
                                    ins=[nd_ins[ch].opt()],
                                    outs=[nd_outs[ch].opt()])
                            else:
                                nc.sync.dma_start(nd_outs[ch][:, :],
                                                  nd_ins[ch][:, :])

                        def sub_i(jc):
                            # eb/backward side: independent of the colmin
                            # AllReduce, runs LAG windows ahead
                            accb = [baccp.tile([8, 512], F32, tag=f"accb{q}",
                                               name=f"accb{q}")
                                    for q in range(nq)]
                            d2bs = []
                            for t in range(NT):
                                psd = bpsd.tile([128, B_W], F32, tag="psd")
                                for g in range(NGB):
                                    q0 = g * 512
                                    nc.tensor.matmul(
                                        psd[:, q0:q0 + 512],
                                        lhsT=t5b[32 * g:32 * g + 5,
                                                 t * 128:(t + 1) * 128],
                                        rhs=c5b[32 * g:32 * g + 5,
                                                jc * B_W + q0:
                                                jc * B_W + q0 + 512],
                                        start=True, stop=True,
                                        tile_position=(32 * g, 0))
                                d2b = bd2.tile([128, B_W], BF16, tag="d2b")
                                nc.scalar.activation(d2b[:], psd[:], ACTF.Relu)
                                d2bs.append(d2b)
                            for t in range(NT):
                                eb = bep.tile([128, B_W], BF16, tag="eb")
                                nc.vector.tensor_scalar(eb[:], d2bs[t][:],
                                                        m_relu[:, t:t + 1],
                                                        None, op0=ALU.is_equal)
                                for q in range(nq):
                                    nc.tensor.matmul(
                                        accb[q][:, :],
                                        lhsT=wb_all[:, t * 8:(t + 1) * 8],
                                        rhs=eb[:, q * 512:(q + 1) * 512],
                                        start=(t == 0), stop=(t == NT - 1))
                            ch, col = divmod(jc, jpc)
                            for q in range(nq):
                                j0 = col * B_W + q * 512
                                ndb = bnd.tile([8, 512], F32, tag="ndb")
                                nc.scalar.copy(ndb[:], accb[q][:, :])
                                nc.sync.dma_start(nd_ins[ch][0:8, j0:j0 + 512],
                                                  ndb[:])
                            return d2bs

                        def sub_ii(jc, d2bs):
                            # ef/forward side vs global colmin (needs the
                            # colmin AllReduce, hidden behind sub_i's lead)
                            m2w = bm2b.tile([1, B_W], F32, tag="m2w")
                            nc.sync.dma_start(
                                m2w[:], m2_out[:, jc * B_W:(jc + 1) * B_W])
                            m2wb = bm2b.tile([1, B_W], BF16, tag="m2wb")
                            nc.vector.tensor_copy(m2wb[:], m2w[:])
                            m2sl = bm2b.tile([128, B_W], BF16, tag="m2b")
                            nc.gpsimd.partition_broadcast(m2sl[:], m2wb[:])
                            accf = [baccp.tile([4, 512], F32, tag=f"accf{q}",
                                               name=f"accf{q}")
                                    for q in range(nq)]
                            for t in range(NT):
                                ef = bep.tile([128, B_W], BF16, tag="ef")
                                nc.vector.tensor_tensor(ef[:], d2bs[t][:],
                                                        m2sl[:], op=ALU.is_le)
                                for q in range(nq):
                                    nc.tensor.matmul(
                                        accf[q][:, :],
                                        lhsT=wf_all[:, t * 4:(t + 1) * 4],
                                        rhs=ef[:, q * 512:(q + 1) * 512],
                                        start=(t == 0), stop=(t == NT - 1))
                            ch, col = divmod(jc, jpc)
                            for q in range(nq):
                                j0 = col * B_W + q * 512
                                ndf = bnd.tile([4, 512], F32, tag="ndf")
                                nc.vector.tensor_copy(ndf[:], accf[q][:, :])
                                nc.sync.dma_start(nd_ins[ch][8:12, j0:j0 + 512],
                                                  ndf[:])
                            if (jc + 1) % jpc == 0:
                                reduce_chunk(jc // jpc)

                        pend = {}
                        for jc in range(njc):
                            pend[jc] = sub_i(jc)
                            if jc >= LAG:
                                sub_ii(jc - LAG, pend.pop(jc - LAG))
                        for jc in range(njc - LAG, njc):
                            sub_ii(jc, pend.pop(jc))

                if "FIN" in phases:
                    # ---- finalize, chunked by nd AllReduce chunk (chunk 0
                    # runs while chunk 1's collective is in flight) ----------
                    prow = lch // lpf  # plane partitions per nd chunk
                    with (
                        tc.tile_pool(name="fin", bufs=1) as fp,
                        tc.tile_pool(name="fin_ps", bufs=1, space="PSUM") as fps,
                    ):
                        accp = fp.tile([prow, NDCH], F32, tag="accp")
                        for ch in range(NDCH):
                            j0 = ch * lch

                            def plane(dram_row, tg):
                                tl = fp.tile([prow, lpf], F32, tag=tg,
                                             name=f"{tg}_{ch}")
                                nc.sync.dma_start(
                                    tl[:], dram_row.rearrange("(p q) -> p q",
                                                              p=prow))
                                return tl

                            def plane_nd(k, tg):
                                tl = fp.tile([prow, lpf], F32, tag=tg,
                                             name=f"{tg}_{ch}")
                                nc.sync.dma_start(
                                    tl[:], nd_outs[ch][k, :].rearrange(
                                        "(p q) -> p q", p=prow))
                                return tl

                            rgbp = [plane(rgbpd[k, j0:j0 + lch], f"rgb{k}")
                                    for k in range(3)]
                            keepf = plane(keepd[0, j0:j0 + lch], "keepf")
                            nd = [plane_nd(k, f"nd{k}") for k in range(12)]

                            num, den = nd[0:3], nd[3]
                            s0, cnt0 = nd[4:7], nd[7]
                            sf, cntf = nd[8:11], nd[11]

                            _cnt = [0]

                            def newt():
                                _cnt[0] += 1
                                return fp.tile([prow, lpf], F32,
                                               tag=f"fin{_cnt[0]}",
                                               name=f"fin{_cnt[0]}_{ch}")

                            dsafe = newt()
                            nc.vector.tensor_scalar(dsafe[:], den[:], 0.0, None,
                                                    op0=ALU.is_equal)
                            nc.vector.tensor_tensor(dsafe[:], dsafe[:], den[:],
                                                    op=ALU.add)
                            rden = newt()
                            nc.vector.reciprocal(rden[:], dsafe[:])
                            c0safe = newt()
                            nc.vector.tensor_scalar(c0safe[:], cnt0[:], 0.0, None,
                                                    op0=ALU.is_equal)
                            nc.vector.tensor_tensor(c0safe[:], c0safe[:], cnt0[:],
                                                    op=ALU.add)
                            rcnt0 = newt()
                            nc.vector.reciprocal(rcnt0[:], c0safe[:])
                            rcntf = newt()
                            nc.vector.reciprocal(rcntf[:], cntf[:])

                            mden = fp.tile([prow, lpf], mybir.dt.int32,
                                           tag="mden", name=f"mden_{ch}")
                            nc.vector.tensor_scalar(mden[:], den[:], 0.0, None,
                                                    op0=ALU.not_equal)
                            mz = fp.tile([prow, lpf], mybir.dt.int32, tag="mz",
                                         name=f"mz_{ch}")
                            nc.vector.tensor_scalar(mz[:], cnt0[:], 0.0, None,
                                                    op0=ALU.is_gt)

                            acc = newt()
                            nc.vector.memset(acc[:], 0.0)
                            for c in range(3):
                                rec = newt()
                                nc.vector.tensor_tensor(rec[:], sf[c][:],
                                                        rcntf[:], op=ALU.mult)
                                tmp = newt()
                                nc.vector.tensor_tensor(tmp[:], num[c][:],
                                                        rden[:], op=ALU.mult)
                                nc.vector.copy_predicated(rec[:], mden[:], tmp[:])
                                nc.vector.tensor_tensor(tmp[:], s0[c][:],
                                                        rcnt0[:], op=ALU.mult)
                                nc.vector.copy_predicated(rec[:], mz[:], tmp[:])
                                diff = newt()
                                nc.vector.tensor_tensor(diff[:], rgbp[c][:],
                                                        rec[:], op=ALU.subtract)
                                ad = newt()
                                nc.scalar.activation(ad[:], diff[:], ACTF.Abs)
                                nc.vector.tensor_tensor(acc[:], acc[:], ad[:],
                                                        op=ALU.add)
                            nc.vector.tensor_tensor(acc[:], acc[:], keepf[:],
                                                    op=ALU.mult)
                            nc.vector.tensor_reduce(accp[:, ch:ch + 1], acc[:],
                                                    axis=AX.X, op=ALU.add)

                        onescol = fp.tile([128, 1], F32, tag="onescol")
                        nc.vector.memset(onescol[:], 1.0)
                        ps_a = fps.tile([1, 1], F32, tag="ps_a")
                        nc.tensor.matmul(ps_a[:], lhsT=onescol[:],
                                         rhs=rowsbce[:], start=True, stop=True)
                        ps_b = fps.tile([1, NDCH], F32, tag="ps_b")
                        nc.tensor.matmul(ps_b[:], lhsT=onescol[0:prow, :],
                                         rhs=accp[:], start=True, stop=True)
                        chsb = fp.tile([1, 2], F32, tag="chsb")
                        nc.sync.dma_start(chsb[:], chaind[:, :])
                        nc.vector.tensor_scalar(chsb[:], chsb[:], 0.0, None,
                                                op0=ALU.mult)
                        outsb = fp.tile([1, 2], F32, tag="outsb")
                        nc.scalar.copy(outsb[:, 0:1], ps_a[:])
                        nc.vector.tensor_reduce(outsb[:, 1:2], ps_b[:],
                                                axis=AX.X, op=ALU.add)
                        nc.vector.tensor_tensor(outsb[:], outsb[:], chsb[:],
                                                op=ALU.add)
                        nc.sync.dma_start(outd[:, :], outsb[:])

    nc.compile()
    return nc


def _host_prep(pred_F, cand_xyz, cand_rgb, tgt_xyz, tgt_rgb, keep_target,
               points_num):
    bf16 = mybir.dt.np(BF16)
    nsh = N // NCORES
    npad = NT * 128
    pred = np.ascontiguousarray(np.asarray(pred_F, np.float32))
    cxyz = np.ascontiguousarray(np.asarray(cand_xyz, np.float32))
    crgb = np.ascontiguousarray(np.asarray(cand_rgb, np.float32))
    txyz = np.ascontiguousarray(np.asarray(tgt_xyz, np.float32))
    trgb_np = np.ascontiguousarray(np.asarray(tgt_rgb, np.float32))
    ktgt = np.asarray(keep_target).astype(np.float32)

    # keep mask (exact reference semantics, f32)
    Lfull = pred.shape[0]
    p8 = pred.reshape(-1, 8)
    rows = np.arange(p8.shape[0])
    ilm = np.zeros(p8.shape, dtype=bool)
    ilm[rows, np.argmax(p8, axis=1)] = True
    ilm = ilm.reshape(-1)
    k = Lfull - int(points_num)
    vals = np.where(ilm, np.inf, pred)
    thr = np.sort(vals)[k - 1]
    keep = (pred > thr) | ilm

    # compact kept candidates to LK columns (pad: b2=BIG never wins a min,
    # keepf=0 never reaches the loss)
    kidx = np.nonzero(keep)[0]
    nk = kidx.size
    assert nk <= LK, f"kept={nk} > {LK}"
    cxyz_k = np.zeros((LK, 3), np.float32)
    cxyz_k[:nk] = cxyz[kidx]
    b2 = np.sum(cxyz * cxyz, axis=1, dtype=np.float32).astype(np.float32)
    b2k = np.full((LK,), BIG, np.float32)
    b2k[:nk] = b2[kidx]
    ones = np.ones(LK, np.float32)
    c5_np = np.stack([cxyz_k[:, 0], cxyz_k[:, 1], cxyz_k[:, 2], ones, b2k])
    c5_np = np.ascontiguousarray(c5_np.astype(bf16))

    rgbp = np.zeros((3, LK), np.float32)
    rgbp[:, :nk] = (crgb[kidx] * np.float32(255.0)).T
    rgbp = np.ascontiguousarray(rgbp)
    keepf = np.zeros((1, LK), np.float32)
    keepf[0, :nk] = 1.0

    a2 = np.sum(txyz * txyz, axis=1, dtype=np.float32).astype(np.float32)

    t5_cores, trgb_cores = [], []
    for c in range(NCORES):
        sl = slice(c * nsh, (c + 1) * nsh)
        t5 = np.zeros((5, npad), np.float32)
        t5[3, :] = BIG     # pad rows: s = 1e30 everywhere
        t5[4, :] = 1.0
        t5[0, :nsh] = -2.0 * txyz[sl, 0]
        t5[1, :nsh] = -2.0 * txyz[sl, 1]
        t5[2, :nsh] = -2.0 * txyz[sl, 2]
        t5[3, :nsh] = a2[sl]
        tr = np.zeros((npad, 3), np.float32)
        tr[:nsh] = trgb_np[sl]
        # [p, t*3+c] layout: target i_local = t*128 + p
        trc = tr.reshape(NT, 128, 3).transpose(1, 0, 2).reshape(128, NT * 3)
        t5_cores.append(np.ascontiguousarray(t5.astype(bf16)))
        trgb_cores.append(np.ascontiguousarray(trc))

    eye = np.eye(128, dtype=np.float32)

    common = dict(c5r=c5_np, rgbp=rgbp,
                  keepf=keepf, predf=pred.reshape(1, Lfull),
                  ktgt=ktgt.reshape(1, Lfull), eye128=eye,
                  chain=np.zeros((1, 2), np.float32))
    in_maps = [dict(common, t5=t5_cores[c], trgb=trgb_cores[c])
               for c in range(NCORES)]
    return in_maps


_CACHE = {}


def kernel(pred_F, cand_xyz, cand_rgb, tgt_xyz, tgt_rgb, keep_target,
           points_num=8192, **_ignored):
    in_maps = _host_prep(pred_F, cand_xyz, cand_rgb, tgt_xyz, tgt_rgb,
                         keep_target, points_num)
    if "nc" not in _CACHE:
        _CACHE["nc"] = _build_nc()
    res = run_bass_kernel_spmd(_CACHE["nc"], in_maps,
                               core_ids=list(range(NCORES)))
    return np.asarray(res.results[0]["out"], np.float32).reshape(2)


if __name__ == "__main__":
    import reference as R
    inputs = R.setup_inputs()
    inputs = {kk: np.asarray(vv) if not np.isscalar(vv) else vv
              for kk, vv in inputs.items()}
    out = kernel(**inputs)
    print("kernel out:", out)
